# revision 1
# baseline (speedup 1.0000x reference)
"""Trainium2 Bass kernel for nn_MessagePassingNN (gnn_message_passing).

B, N, F, H, A, T = 4, 256, 64, 256, 16, 3

Sharding: 8 cores = (batch b, receiver-half). Core c handles batch c//2 and
receiver nodes [128*(c%2), 128*(c%2+1)). All node indexing inside the kernel is
core-RELATIVE ([my 128 | partner 128]) so the SPMD program is identical on all
cores; the host permutes the inputs per core.

Math (per message-passing iteration):
    e[i,j,:] = relu(hi[i,:] + hj[j,:] + b1)           (hi = h@W1_i, hj = h@W1_j)
    agg[i,:] = sum_j adj[i,j] * e[i,j,:] @ W2 + deg[i]*b2     <- linearity trick:
        the W2 matmul is pulled OUT of the j-sum (34 GFLOP -> 134 MFLOP).
    GRU update on agg/h.

Device layout: everything transposed [feature-on-partitions, node-on-free].
Hot loop, [h-partitions, j-free] tiles, bf16 with f32 accumulation:
    1) mask-inject (DVE TT, batched IB receivers via a stride-0 free dim):
       w = hjbT + adj_bc, where adj_bc holds 32*(adj-1) in {-32, 0} so
       masked entries die after the relu (exact for adj=1).
    2) relu+bias+reduce, one op per (receiver, h-tile), split ScalarE/VectorE:
       ACT: activation(Relu, bias=hiT[:,i], accum_out=aggT[:,i])
       DVE: scalar_tensor_tensor((w+hi) max 0, accum_out=aggT[:,i])
adj_bc is the adjacency row broadcast across 128 partitions (built once by
stride-0 DMAs; iteration-invariant).

h-halves are exchanged per iteration with a pairwise AllReduce(add) of the
update + a subtract of our own contribution (static addressing on all cores).
"""

import sys

sys.path.insert(0, "/opt/trn_rl_repo")

import numpy as np

import concourse.bass as bass
import concourse.bacc as bacc
import concourse.tile as tile
from concourse import mybir
from concourse.bass_utils import run_bass_kernel_spmd

B, N, F, H, A, T = 4, 256, 64, 256, 16, 3
NLOC = 128          # receivers per core
HT = H // 128       # h-dim tiles (2)
f32 = mybir.dt.float32
bf16 = mybir.dt.bfloat16
BF16_NP = mybir.dt.np(bf16)

USE_BF16 = True     # e-tensor path in bf16 (accumulate in f32)
IB = 4              # receivers per batched mask-inject op
ACT_OF_16 = 9       # of every 16 (i,ht) pairs, this many accumulate on ScalarE
GPS_OF_3 = 0        # of every 3 mask-inject groups, this many run on GpSimd

_CACHE = {}


def _mm_acc(nc, ps, w_sb, m_off, rhs_tiles, kt_count, extra=None):
    """psum ps[:, :] = sum_kt W[kt, m_off:m_off+mw].T @ rhs_tiles[kt]; extra =
    optional (lhsT, rhs) accumulated at the end."""
    n_ins = kt_count + (1 if extra is not None else 0)
    idx = 0
    for kt in range(kt_count):
        lhsT = w_sb[:, kt * w_sb.mcols + m_off: kt * w_sb.mcols + m_off + ps.shape[0]]
        nc.tensor.matmul(ps, lhsT, rhs_tiles[kt], start=(idx == 0), stop=(idx == n_ins - 1))
        idx += 1
    if extra is not None:
        lhsT, rhs = extra
        nc.tensor.matmul(ps, lhsT, rhs, start=False, stop=True)


class _WSb:
    """SBUF weight holder: W [K, M] stored as [128, (K//128)*M]."""

    def __init__(self, nc, pool, dram, K, M, name):
        self.mcols = M
        self.kt = K // 128
        self.sb = pool.tile([128, self.kt * M], f32, name=name, tag=name)
        for kt in range(self.kt):
            nc.sync.dma_start(
                out=self.sb[:, kt * M:(kt + 1) * M],
                in_=dram[kt * 128:(kt + 1) * 128, :],
            )

    def __getitem__(self, sl):
        return self.sb[sl]


def build_program():
    nc = bacc.Bacc("TRN2", target_bir_lowering=False, debug=False, num_devices=8)

    # ---------------- I/O ----------------
    xT_d = nc.dram_tensor("xT", [F, N], f32, kind="ExternalInput")
    adj_d = nc.dram_tensor("adjb", [NLOC, N], bf16, kind="ExternalInput")
    w_pre1 = nc.dram_tensor("pre_W1", [F, H], f32, kind="ExternalInput")
    w_pre2 = nc.dram_tensor("pre_W2", [H, H], f32, kind="ExternalInput")
    w_m1i = nc.dram_tensor("W1i", [H, H], f32, kind="ExternalInput")
    w_m1j = nc.dram_tensor("W1j", [H, H], f32, kind="ExternalInput")
    w_m2 = nc.dram_tensor("W2m", [H, H], f32, kind="ExternalInput")
    w_ih = nc.dram_tensor("Wih", [H, 3 * H], f32, kind="ExternalInput")
    w_hh = nc.dram_tensor("Whh", [H, 3 * H], f32, kind="ExternalInput")
    w_ro1 = nc.dram_tensor("roW1", [H, H], f32, kind="ExternalInput")
    w_ro2 = nc.dram_tensor("roW2", [H, A], f32, kind="ExternalInput")
    preb1_d = nc.dram_tensor("preb1c", [128, HT], f32, kind="ExternalInput")
    preb2_d = nc.dram_tensor("preb2c", [128, HT], f32, kind="ExternalInput")
    msgb1_d = nc.dram_tensor("msgb1c", [128, HT], f32, kind="ExternalInput")
    msgb2_d = nc.dram_tensor("msgb2r", [1, H], f32, kind="ExternalInput")
    brz_d = nc.dram_tensor("brzc", [128, 4], f32, kind="ExternalInput")
    bihn_d = nc.dram_tensor("bihnc", [128, HT], f32, kind="ExternalInput")
    bhhn_d = nc.dram_tensor("bhhnc", [128, HT], f32, kind="ExternalInput")
    rob1_d = nc.dram_tensor("rob1c", [128, HT], f32, kind="ExternalInput")
    rob2_d = nc.dram_tensor("rob2c", [A, 1], f32, kind="ExternalInput")
    ident_d = nc.dram_tensor("ident", [128, 128], f32, kind="ExternalInput")
    q_out = nc.dram_tensor("q_out", [A, 1], f32, kind="ExternalOutput")

    # collective bounce buffers (one pair per iteration)
    cc_in = [nc.dram_tensor(f"cc_in_{t}", [H, NLOC], f32) for t in range(T)]
    cc_out = [nc.dram_tensor(f"cc_out_{t}", [H, NLOC], f32) for t in range(T)]
    groups = [[0, 1], [2, 3], [4, 5], [6, 7]]

    e_dt = bf16 if USE_BF16 else f32

    with tile.TileContext(nc) as tc:
        import contextlib

        with contextlib.ExitStack() as ctx:
            singles = ctx.enter_context(tc.tile_pool(name="singles", bufs=1))
            work = ctx.enter_context(tc.tile_pool(name="work", bufs=3))
            eloop = ctx.enter_context(tc.tile_pool(name="eloop", bufs=8))
            psp = ctx.enter_context(tc.tile_pool(name="psp", bufs=6, space="PSUM"))

            # ---------------- weights/constants to SBUF ----------------
            # critical path first: preprocess inputs, then adjacency chunks,
            # then the remaining (larger) weights.
            xT_sb = work.tile([F, N], f32, name="xT_sb", tag="xT_sb")
            nc.sync.dma_start(out=xT_sb[:], in_=xT_d[:])
            # pre_W1 has K=F=64 (single sub-128 contraction tile)
            Wpre1_sb = singles.tile([F, H], f32)
            nc.sync.dma_start(out=Wpre1_sb[:], in_=w_pre1[:])
            W_pre2 = _WSb(nc, singles, w_pre2[:], H, H, "Wpre2")
            W_m1i = _WSb(nc, singles, w_m1i[:], H, H, "Wm1i")
            W_m1j = _WSb(nc, singles, w_m1j[:], H, H, "Wm1j")
            # adjacency rows (bf16) + broadcast across partitions
            adj_sb = singles.tile([NLOC, N], bf16)
            nc.sync.dma_start(out=adj_sb[:], in_=adj_d[:])
            CH = 8  # broadcast chunks
            rows_per = NLOC // CH
            adj_bct = [
                singles.tile([128, rows_per * N], e_dt, name=f"adjbc{c}", tag=f"adjbc{c}")
                for c in range(CH)
            ]
            for c in range(CH):
                bc_in = bass.AP(
                    tensor=adj_d,
                    offset=c * rows_per * N,
                    ap=[[0, 128], [N, rows_per], [1, N]],
                )
                eng = nc.gpsimd if c % 2 == 0 else nc.sync
                eng.dma_start(out=adj_bct[c][:], in_=bc_in)

            W_m2 = _WSb(nc, singles, w_m2[:], H, H, "Wm2")
            W_ih = _WSb(nc, singles, w_ih[:], H, 3 * H, "Wih")
            W_hh = _WSb(nc, singles, w_hh[:], H, 3 * H, "Whh")
            W_ro1 = _WSb(nc, singles, w_ro1[:], H, H, "Wro1")
            W_ro2 = _WSb(nc, singles, w_ro2[:], H, A, "Wro2")

            def _load(shape, dram, name):
                t_ = singles.tile(list(shape), f32, name=name, tag=name)
                nc.sync.dma_start(out=t_[:], in_=dram[:])
                return t_

            preb1 = _load([128, HT], preb1_d, "preb1")
            preb2 = _load([128, HT], preb2_d, "preb2")
            msgb1 = _load([128, HT], msgb1_d, "msgb1")
            msgb2 = _load([1, H], msgb2_d, "msgb2")
            brz = _load([128, 4], brz_d, "brz")
            bihn = _load([128, HT], bihn_d, "bihn")
            bhhn = _load([128, HT], bhhn_d, "bhhn")
            rob1 = _load([128, HT], rob1_d, "rob1")
            rob2 = _load([A, 1], rob2_d, "rob2")
            ident = _load([128, 128], ident_d, "ident")

            zeros_e = singles.tile([128, N], e_dt)
            nc.vector.memset(zeros_e[:], 0.0)

            # degree: adj_sb holds 32*(adj-1) -> deg = reduce/32 + N
            deg_col = singles.tile([NLOC, 1], f32)
            nc.vector.reduce_sum(deg_col[:], adj_sb[:], axis=mybir.AxisListType.X)
            nc.vector.tensor_scalar(
                deg_col[:], deg_col[:], 1.0 / 32.0, float(N),
                mybir.AluOpType.mult, mybir.AluOpType.add,
            )
            ps_t = psp.tile([128, 512], f32, name="ps", tag="ps")
            nc.tensor.transpose(ps_t[0:1, 0:NLOC], deg_col[:], ident[:])
            deg_row = singles.tile([1, NLOC], f32)
            nc.vector.tensor_copy(deg_row[:], ps_t[0:1, 0:NLOC])

            # ---------------- preprocess: h0 ----------------
            hT = [singles.tile([128, N], f32, name=f"hT{ht}", tag=f"hT{ht}") for ht in range(HT)]
            p1 = [work.tile([128, N], f32, name=f"p1_{ht}", tag=f"p1_{ht}") for ht in range(HT)]
            for ht in range(HT):
                ps = psp.tile([128, 512], f32, name="ps", tag="ps")
                nc.tensor.matmul(
                    ps[:, 0:N], Wpre1_sb[:, ht * 128:(ht + 1) * 128], xT_sb[:],
                    start=True, stop=True,
                )
                nc.scalar.activation(
                    p1[ht][:], ps[:, 0:N], mybir.ActivationFunctionType.Relu,
                    bias=preb1[:, ht:ht + 1],
                )
            for ht in range(HT):
                ps = psp.tile([128, 512], f32, name="ps", tag="ps")
                _mm_acc(nc, ps[:, 0:N], W_pre2, ht * 128, p1, HT)
                nc.scalar.activation(
                    hT[ht][:], ps[:, 0:N], mybir.ActivationFunctionType.Identity,
                    bias=preb2[:, ht:ht + 1],
                )

            # ---------------- message passing iterations ----------------
            for t in range(T):
                # hjbT = (h @ W1_j + b1).T  -> [h, j] layout, bf16
                hjbT = [work.tile([128, N], e_dt, name=f"hjbT{ht}", tag=f"hjbT{ht}") for ht in range(HT)]
                for ht in range(HT):
                    ps = psp.tile([128, 512], f32, name="ps", tag="ps")
                    _mm_acc(nc, ps[:, 0:N], W_m1j, ht * 128, hT, HT)
                    nc.scalar.activation(
                        hjbT[ht][:], ps[:, 0:N],
                        mybir.ActivationFunctionType.Identity,
                        bias=msgb1[:, ht:ht + 1],
                    )
                # hiT = (h_loc @ W1_i).T -> [h, i] layout (f32: TS scalar must be f32)
                hiTf = [work.tile([128, NLOC], f32, name=f"hiTf{ht}", tag=f"hiTf{ht}") for ht in range(HT)]
                for ht in range(HT):
                    ps = psp.tile([128, 512], f32, name="ps", tag="ps")
                    _mm_acc(
                        nc, ps[:, 0:NLOC], W_m1i, ht * 128,
                        [h_[:, 0:NLOC] for h_ in hT], HT,
                    )
                    nc.vector.tensor_copy(hiTf[ht][:], ps[:, 0:NLOC])

                # e-loop. adj_bc holds 32*adj; w = (hjbT - 32) + 32*adj kills
                # masked (adj=0) entries after the relu. Mask-inject is a plain
                # TT add of (hjbT-32) + 32adj batched over IB receivers, split
                # DVE/GpSimd; relu+bias+accumulate is one op per (i, h-tile),
                # split ScalarE/VectorE.
                aggT = [work.tile([128, NLOC], f32, name=f"aggT{ht}", tag=f"aggT{ht}") for ht in range(HT)]
                gpc = NLOC // IB // CH  # groups per broadcast chunk
                for ht in range(HT):
                    hjb_rep = bass.AP(
                        tensor=hjbT[ht].tensor, offset=hjbT[ht].offset,
                        ap=[hjbT[ht].ap[0], [0, IB], [1, N]],
                    )
                    for g in range(NLOC // IB):
                        w = eloop.tile([128, IB * N], e_dt, name="w", tag="w")
                        nc.vector.tensor_tensor(
                            out=w[:], in0=hjb_rep,
                            in1=adj_bct[g // gpc][:, (g % gpc) * IB * N:(g % gpc + 1) * IB * N],
                            op=mybir.AluOpType.add,
                        )
                        for k in range(IB):
                            i = g * IB + k
                            scr = eloop.tile([128, N], e_dt, name="scr", tag="scr")
                            if (i % 16) < ACT_OF_16:
                                nc.scalar.activation(
                                    scr[:], w[:, k * N:(k + 1) * N],
                                    mybir.ActivationFunctionType.Relu,
                                    bias=hiTf[ht][:, i:i + 1],
                                    accum_out=aggT[ht][:, i:i + 1],
                                )
                            else:
                                nc.vector.scalar_tensor_tensor(
                                    out=scr[:], in0=w[:, k * N:(k + 1) * N],
                                    scalar=hiTf[ht][:, i:i + 1],
                                    in1=zeros_e[:],
                                    op0=mybir.AluOpType.add,
                                    op1=mybir.AluOpType.max,
                                    accum_out=aggT[ht][:, i:i + 1],
                                )

                # msgT = W2.T @ aggT + outer(b2, deg)
                msgT = [work.tile([128, NLOC], f32, name=f"msgT{ht}", tag=f"msgT{ht}") for ht in range(HT)]
                for ht in range(HT):
                    ps = psp.tile([128, 512], f32, name="ps", tag="ps")
                    _mm_acc(
                        nc, ps[:, 0:NLOC], W_m2, ht * 128, aggT, HT,
                        extra=(msgb2[0:1, ht * 128:(ht + 1) * 128], deg_row[:]),
                    )
                    nc.vector.tensor_copy(msgT[ht][:], ps[:, 0:NLOC])

                # GRU gates (torch order r, z, n); everything [gate-dim, i]
                hloc = [h_[:, 0:NLOC] for h_ in hT]
                ps_rz = psp.tile([128, 512], f32, name="ps", tag="ps")
                for mt in range(4):  # r0 r1 z0 z1
                    for kt in range(HT):
                        nc.tensor.matmul(
                            ps_rz[:, mt * 128:(mt + 1) * 128],
                            W_ih[:, kt * 768 + mt * 128: kt * 768 + (mt + 1) * 128],
                            msgT[kt][:], start=(kt == 0), stop=False,
                        )
                    for kt in range(HT):
                        nc.tensor.matmul(
                            ps_rz[:, mt * 128:(mt + 1) * 128],
                            W_hh[:, kt * 768 + mt * 128: kt * 768 + (mt + 1) * 128],
                            hloc[kt], start=False, stop=(kt == HT - 1),
                        )
                rz = work.tile([128, 512], f32, name="rz", tag="rz")
                for mt in range(4):
                    nc.scalar.activation(
                        rz[:, mt * 128:(mt + 1) * 128],
                        ps_rz[:, mt * 128:(mt + 1) * 128],
                        mybir.ActivationFunctionType.Sigmoid,
                        bias=brz[:, mt:mt + 1],
                    )
                # n = tanh(gi_n + bihn + r * (gh_n + bhhn))
                ps_gin = psp.tile([128, 512], f32, name="ps", tag="ps")
                ps_hn = psp.tile([128, 512], f32, name="ps", tag="ps")
                for ht in range(HT):
                    for kt in range(HT):
                        nc.tensor.matmul(
                            ps_gin[:, ht * 128:(ht + 1) * 128],
                            W_ih[:, kt * 768 + (4 + ht) * 128: kt * 768 + (5 + ht) * 128],
                            msgT[kt][:], start=(kt == 0), stop=(kt == HT - 1),
                        )
                        nc.tensor.matmul(
                            ps_hn[:, ht * 128:(ht + 1) * 128],
                            W_hh[:, kt * 768 + (4 + ht) * 128: kt * 768 + (5 + ht) * 128],
                            hloc[kt], start=(kt == 0), stop=(kt == HT - 1),
                        )
                hnew = [work.tile([128, NLOC], f32, name=f"hnew{ht}", tag=f"hnew{ht}") for ht in range(HT)]
                for ht in range(HT):
                    hn = work.tile([128, NLOC], f32, name="hn", tag="hn")
                    nc.scalar.activation(
                        hn[:], ps_hn[:, ht * 128:(ht + 1) * 128],
                        mybir.ActivationFunctionType.Identity,
                        bias=bhhn[:, ht:ht + 1],
                    )
                    rhn = work.tile([128, NLOC], f32, name="rhn", tag="rhn")
                    nc.vector.tensor_mul(rhn[:], rz[:, ht * 128:(ht + 1) * 128], hn[:])
                    nsum = work.tile([128, NLOC], f32, name="nsum", tag="nsum")
                    nc.vector.tensor_add(
                        nsum[:], rhn[:], ps_gin[:, ht * 128:(ht + 1) * 128]
                    )
                    n_t = work.tile([128, NLOC], f32, name="n_t", tag="n_t")
                    nc.scalar.activation(
                        n_t[:], nsum[:], mybir.ActivationFunctionType.Tanh,
                        bias=bihn[:, ht:ht + 1],
                    )
                    # h' = n + z*(h - n)
                    hmn = work.tile([128, NLOC], f32, name="hmn", tag="hmn")
                    nc.vector.tensor_sub(hmn[:], hloc[ht], n_t[:])
                    zh = work.tile([128, NLOC], f32, name="zh", tag="zh")
                    nc.vector.tensor_mul(zh[:], rz[:, 256 + ht * 128: 256 + (ht + 1) * 128], hmn[:])
                    nc.vector.tensor_add(hnew[ht][:], n_t[:], zh[:])

                # exchange: AllReduce(pair) of my update; partner = sum - mine
                for ht in range(HT):
                    nc.sync.dma_start(
                        out=cc_in[t][ht * 128:(ht + 1) * 128, :], in_=hnew[ht][:]
                    )
                nc.gpsimd.collective_compute(
                    "AllReduce", mybir.AluOpType.add, replica_groups=groups,
                    ins=[cc_in[t][:]], outs=[cc_out[t][:]],
                )
                for ht in range(HT):
                    rem = work.tile([128, NLOC], f32, name="rem", tag="rem")
                    nc.sync.dma_start(
                        out=rem[:], in_=cc_out[t][ht * 128:(ht + 1) * 128, :]
                    )
                    nc.vector.tensor_copy(hT[ht][:, 0:NLOC], hnew[ht][:])
                    nc.vector.tensor_sub(hT[ht][:, NLOC:N], rem[:], hnew[ht][:])

            # ---------------- readout ----------------
            gT = [work.tile([128, 1], f32, name=f"gT{ht}", tag=f"gT{ht}") for ht in range(HT)]
            for ht in range(HT):
                nc.vector.reduce_sum(gT[ht][:], hT[ht][:], axis=mybir.AxisListType.X)
            y1 = [work.tile([128, 1], f32, name=f"y1{ht}", tag=f"y1{ht}") for ht in range(HT)]
            for ht in range(HT):
                ps = psp.tile([128, 512], f32, name="ps", tag="ps")
                _mm_acc(nc, ps[:, 0:1], W_ro1, ht * 128, gT, HT)
                nc.scalar.activation(
                    y1[ht][:], ps[:, 0:1], mybir.ActivationFunctionType.Relu,
                    bias=rob1[:, ht:ht + 1],
                )
            ps_q = psp.tile([128, 512], f32, name="ps", tag="ps")
            for kt in range(HT):
                nc.tensor.matmul(
                    ps_q[0:A, 0:1], W_ro2[:, kt * A:(kt + 1) * A], y1[kt][:],
                    start=(kt == 0), stop=(kt == HT - 1),
                )
            q_sb = work.tile([A, 1], f32, name="q_sb", tag="q_sb")
            nc.scalar.activation(
                q_sb[:], ps_q[0:A, 0:1], mybir.ActivationFunctionType.Identity,
                bias=rob2[:],
            )
            nc.sync.dma_start(out=q_out[:], in_=q_sb[:])

    nc.compile()
    return nc


def _in_maps(inputs):
    nf = np.asarray(inputs["node_features"], np.float32)
    adj = np.asarray(inputs["adjacency"])
    msg_W1 = np.asarray(inputs["msg_W1"], np.float32)
    gbih = np.asarray(inputs["gru_bih"], np.float32)
    gbhh = np.asarray(inputs["gru_bhh"], np.float32)

    def cols(v, nt):  # [nt*128] -> [128, nt] partition-major columns
        return np.ascontiguousarray(np.asarray(v, np.float32).reshape(nt, 128).T)

    shared = {
        "pre_W1": np.asarray(inputs["pre_W1"], np.float32),
        "pre_W2": np.asarray(inputs["pre_W2"], np.float32),
        "W1i": np.ascontiguousarray(msg_W1[:H]),
        "W1j": np.ascontiguousarray(msg_W1[H:]),
        "W2m": np.asarray(inputs["msg_W2"], np.float32),
        "Wih": np.asarray(inputs["gru_Wih"], np.float32),
        "Whh": np.asarray(inputs["gru_Whh"], np.float32),
        "roW1": np.asarray(inputs["ro_W1"], np.float32),
        "roW2": np.asarray(inputs["ro_W2"], np.float32),
        "preb1c": cols(inputs["pre_b1"], HT),
        "preb2c": cols(inputs["pre_b2"], HT),
        "msgb1c": cols(inputs["msg_b1"], HT),
        "msgb2r": np.asarray(inputs["msg_b2"], np.float32)[None, :],
        "brzc": cols((gbih + gbhh)[: 2 * H], 4),
        "bihnc": cols(gbih[2 * H:], HT),
        "bhhnc": cols(gbhh[2 * H:], HT),
        "rob1c": cols(inputs["ro_b1"], HT),
        "rob2c": np.asarray(inputs["ro_b2"], np.float32)[:, None],
        "ident": np.eye(128, dtype=np.float32),
    }
    maps = []
    for c in range(8):
        b, half = c // 2, c % 2
        lo, hi = half * NLOC, (half + 1) * NLOC
        perm = np.r_[lo:hi, 0:lo, hi:N]
        m = dict(shared)
        m["xT"] = np.ascontiguousarray(nf[b].T[:, perm])
        m["adjb"] = np.ascontiguousarray(
            ((adj[b, lo:hi][:, perm] - 1) * 32).astype(BF16_NP)
        )
        maps.append(m)
    return maps


def kernel(**inputs) -> np.ndarray:
    if "nc" not in _CACHE:
        _CACHE["nc"] = build_program()
    nc = _CACHE["nc"]
    maps = _in_maps(inputs)
    res = run_bass_kernel_spmd(nc, maps, list(range(8))).results
    q = np.stack([res[2 * b]["q_out"][:, 0] for b in range(B)]).astype(np.float32)
    return q



# revision 4
# speedup vs baseline: 1.0596x; 1.0596x over previous
"""Trainium2 Bass kernel for nn_MessagePassingNN (gnn_message_passing).

B, N, F, H, A, T = 4, 256, 64, 256, 16, 3

Sharding: 8 cores = (batch b, receiver-half). Core c handles batch c//2 and
receiver nodes [128*(c%2), 128*(c%2+1)). All node indexing inside the kernel is
core-RELATIVE ([my 128 | partner 128]) so the SPMD program is identical on all
cores; the host permutes the inputs per core.

Math (per message-passing iteration):
    e[i,j,:] = relu(hi[i,:] + hj[j,:] + b1)           (hi = h@W1_i, hj = h@W1_j)
    agg[i,:] = sum_j adj[i,j] * e[i,j,:] @ W2 + deg[i]*b2     <- linearity trick:
        the W2 matmul is pulled OUT of the j-sum (34 GFLOP -> 134 MFLOP).
    GRU update on agg/h.

Device layout: everything transposed [feature-on-partitions, node-on-free].
Hot loop, [h-partitions, j-free] tiles, bf16 with f32 accumulation:
    1) mask-inject (DVE TT, batched IB receivers via a stride-0 free dim):
       w = hjbT + adj_bc, where adj_bc holds 32*(adj-1) in {-32, 0} so
       masked entries die after the relu (exact for adj=1).
    2) relu+bias+reduce, one op per (receiver, h-tile), split ScalarE/VectorE:
       ACT: activation(Relu, bias=hiT[:,i], accum_out=aggT[:,i])
       DVE: scalar_tensor_tensor((w+hi) max 0, accum_out=aggT[:,i])
adj_bc is the adjacency row broadcast across 128 partitions (built once by
stride-0 DMAs; iteration-invariant).

h-halves are exchanged per iteration with a pairwise AllReduce(add) of the
update + a subtract of our own contribution (static addressing on all cores).
"""

import sys

sys.path.insert(0, "/opt/trn_rl_repo")

import numpy as np

import concourse.bass as bass
import concourse.bacc as bacc
import concourse.tile as tile
from concourse import mybir
from concourse.bass_utils import run_bass_kernel_spmd

# ---- custom fused DVE op: out = relu(in0 + in1 + s0); accum_out = s1 + sum --
import concourse.dve_ops as dve_ops
from concourse.dve_ops import DveOp
from concourse.dve_spec import Spec, Src0, Src1, C0, C1, relu as _sp_relu, lower
from concourse.dve_spec import AluOp as _SpAluOp
from concourse.dve_uop import DveOpSpec


def _ref_mra(in0, in1, s0, s1, imm2):
    b = np.maximum(in0.astype(np.float32) + in1 + s0, 0.0).astype(np.float32)
    return b, s1 + b.reshape(b.shape[0], -1).sum(axis=-1, keepdims=True)


def _register_fused_op():
    name = "MSG_RELU_ACC_ANT"
    if name in dve_ops._SUB_OPCODE_FOR_NAME:
        return next(o for o in dve_ops.OPS if o.name == name)
    spec = Spec(
        body=_sp_relu(Src0 + Src1 + C0),
        accum=_SpAluOp.ADD,
        accum_init=C1,
        reference=_ref_mra,
    )
    row = max(dve_ops._SUB_OPCODE_FOR_NAME.values()) + 1
    assert row < 0x20
    dve_ops._SUB_OPCODE_FOR_NAME[name] = row
    shas = {}
    for ver in ("v3", "v4"):
        tmp = DveOpSpec(name=name, opcode=row, uops=lower(spec, ver=ver), rd1_en=True)
        shas[ver] = tmp.sha(ver)
    op = DveOp(name, spec, subdim=False, uops_sha=shas)
    dve_ops.OPS.append(op)
    dve_ops.CUSTOM_DVE_SPECS[name] = spec
    return op


MSG_RELU_ACC = _register_fused_op()

B, N, F, H, A, T = 4, 256, 64, 256, 16, 3
NLOC = 128          # receivers per core
HT = H // 128       # h-dim tiles (2)
f32 = mybir.dt.float32
bf16 = mybir.dt.bfloat16
BF16_NP = mybir.dt.np(bf16)

USE_BF16 = True     # e-tensor path in bf16 (accumulate in f32)
SC_OF_16 = 6        # of every 16 (i,ht) pairs, this many accumulate on ScalarE
                    # (premasked ACT path); the rest use the fused DVE op

_CACHE = {}


def _mm_acc(nc, ps, w_sb, m_off, rhs_tiles, kt_count, extra=None):
    """psum ps[:, :] = sum_kt W[kt, m_off:m_off+mw].T @ rhs_tiles[kt]; extra =
    optional (lhsT, rhs) accumulated at the end."""
    n_ins = kt_count + (1 if extra is not None else 0)
    idx = 0
    for kt in range(kt_count):
        lhsT = w_sb[:, kt * w_sb.mcols + m_off: kt * w_sb.mcols + m_off + ps.shape[0]]
        nc.tensor.matmul(ps, lhsT, rhs_tiles[kt], start=(idx == 0), stop=(idx == n_ins - 1))
        idx += 1
    if extra is not None:
        lhsT, rhs = extra
        nc.tensor.matmul(ps, lhsT, rhs, start=False, stop=True)


class _WSb:
    """SBUF weight holder: W [K, M] stored as [128, (K//128)*M]."""

    def __init__(self, nc, pool, dram, K, M, name):
        self.mcols = M
        self.kt = K // 128
        self.sb = pool.tile([128, self.kt * M], f32, name=name, tag=name)
        for kt in range(self.kt):
            nc.sync.dma_start(
                out=self.sb[:, kt * M:(kt + 1) * M],
                in_=dram[kt * 128:(kt + 1) * 128, :],
            )

    def __getitem__(self, sl):
        return self.sb[sl]


def build_program():
    nc = bacc.Bacc("TRN2", target_bir_lowering=False, debug=False, num_devices=8)

    # ---------------- I/O ----------------
    xT_d = nc.dram_tensor("xT", [F, N], f32, kind="ExternalInput")
    adj_d = nc.dram_tensor("adjb", [NLOC, N], bf16, kind="ExternalInput")
    w_pre1 = nc.dram_tensor("pre_W1", [F, H], f32, kind="ExternalInput")
    w_pre2 = nc.dram_tensor("pre_W2", [H, H], f32, kind="ExternalInput")
    w_m1i = nc.dram_tensor("W1i", [H, H], f32, kind="ExternalInput")
    w_m1j = nc.dram_tensor("W1j", [H, H], f32, kind="ExternalInput")
    w_m2 = nc.dram_tensor("W2m", [H, H], f32, kind="ExternalInput")
    w_ih = nc.dram_tensor("Wih", [H, 3 * H], f32, kind="ExternalInput")
    w_hh = nc.dram_tensor("Whh", [H, 3 * H], f32, kind="ExternalInput")
    w_ro1 = nc.dram_tensor("roW1", [H, H], f32, kind="ExternalInput")
    w_ro2 = nc.dram_tensor("roW2", [H, A], f32, kind="ExternalInput")
    preb1_d = nc.dram_tensor("preb1c", [128, HT], f32, kind="ExternalInput")
    preb2_d = nc.dram_tensor("preb2c", [128, HT], f32, kind="ExternalInput")
    msgb1_d = nc.dram_tensor("msgb1c", [128, HT], f32, kind="ExternalInput")
    msgb2_d = nc.dram_tensor("msgb2r", [1, H], f32, kind="ExternalInput")
    brz_d = nc.dram_tensor("brzc", [128, 4], f32, kind="ExternalInput")
    bihn_d = nc.dram_tensor("bihnc", [128, HT], f32, kind="ExternalInput")
    bhhn_d = nc.dram_tensor("bhhnc", [128, HT], f32, kind="ExternalInput")
    rob1_d = nc.dram_tensor("rob1c", [128, HT], f32, kind="ExternalInput")
    rob2_d = nc.dram_tensor("rob2c", [A, 1], f32, kind="ExternalInput")
    ident_d = nc.dram_tensor("ident", [128, 128], f32, kind="ExternalInput")
    q_out = nc.dram_tensor("q_out", [A, 1], f32, kind="ExternalOutput")

    # collective bounce buffers (one pair per iteration)
    cc_in = [nc.dram_tensor(f"cc_in_{t}", [H, NLOC], f32) for t in range(T)]
    cc_out = [nc.dram_tensor(f"cc_out_{t}", [H, NLOC], f32) for t in range(T)]
    groups = [[0, 1], [2, 3], [4, 5], [6, 7]]

    e_dt = bf16 if USE_BF16 else f32

    with tile.TileContext(nc) as tc:
        import contextlib

        with contextlib.ExitStack() as ctx:
            singles = ctx.enter_context(tc.tile_pool(name="singles", bufs=1))
            work = ctx.enter_context(tc.tile_pool(name="work", bufs=3))
            eloop = ctx.enter_context(tc.tile_pool(name="eloop", bufs=8))
            psp = ctx.enter_context(tc.tile_pool(name="psp", bufs=6, space="PSUM"))

            # ---------------- weights/constants to SBUF ----------------
            # critical path first: preprocess inputs, then adjacency chunks,
            # then the remaining (larger) weights.
            xT_sb = work.tile([F, N], f32, name="xT_sb", tag="xT_sb")
            nc.sync.dma_start(out=xT_sb[:], in_=xT_d[:])
            # pre_W1 has K=F=64 (single sub-128 contraction tile)
            Wpre1_sb = singles.tile([F, H], f32)
            nc.sync.dma_start(out=Wpre1_sb[:], in_=w_pre1[:])
            W_pre2 = _WSb(nc, singles, w_pre2[:], H, H, "Wpre2")
            W_m1i = _WSb(nc, singles, w_m1i[:], H, H, "Wm1i")
            W_m1j = _WSb(nc, singles, w_m1j[:], H, H, "Wm1j")
            # adjacency rows (bf16) + broadcast across partitions
            adj_sb = singles.tile([NLOC, N], bf16)
            nc.sync.dma_start(out=adj_sb[:], in_=adj_d[:])
            CH = 8  # broadcast chunks
            rows_per = NLOC // CH
            adj_bct = [
                singles.tile([128, rows_per * N], e_dt, name=f"adjbc{c}", tag=f"adjbc{c}")
                for c in range(CH)
            ]
            for c in range(CH):
                bc_in = bass.AP(
                    tensor=adj_d,
                    offset=c * rows_per * N,
                    ap=[[0, 128], [N, rows_per], [1, N]],
                )
                eng = nc.gpsimd if c % 2 == 0 else nc.sync
                eng.dma_start(out=adj_bct[c][:], in_=bc_in)

            W_m2 = _WSb(nc, singles, w_m2[:], H, H, "Wm2")
            W_ih = _WSb(nc, singles, w_ih[:], H, 3 * H, "Wih")
            W_hh = _WSb(nc, singles, w_hh[:], H, 3 * H, "Whh")
            W_ro1 = _WSb(nc, singles, w_ro1[:], H, H, "Wro1")
            W_ro2 = _WSb(nc, singles, w_ro2[:], H, A, "Wro2")

            def _load(shape, dram, name):
                t_ = singles.tile(list(shape), f32, name=name, tag=name)
                nc.sync.dma_start(out=t_[:], in_=dram[:])
                return t_

            preb1 = _load([128, HT], preb1_d, "preb1")
            preb2 = _load([128, HT], preb2_d, "preb2")
            msgb1 = _load([128, HT], msgb1_d, "msgb1")
            msgb2 = _load([1, H], msgb2_d, "msgb2")
            brz = _load([128, 4], brz_d, "brz")
            bihn = _load([128, HT], bihn_d, "bihn")
            bhhn = _load([128, HT], bhhn_d, "bhhn")
            rob1 = _load([128, HT], rob1_d, "rob1")
            rob2 = _load([A, 1], rob2_d, "rob2")
            ident = _load([128, 128], ident_d, "ident")

            zeros_e = singles.tile([128, N], e_dt)
            nc.vector.memset(zeros_e[:], 0.0)

            # degree: adj_sb holds 32*(adj-1) -> deg = reduce/32 + N
            deg_col = singles.tile([NLOC, 1], f32)
            nc.vector.reduce_sum(deg_col[:], adj_sb[:], axis=mybir.AxisListType.X)
            nc.vector.tensor_scalar(
                deg_col[:], deg_col[:], 1.0 / 32.0, float(N),
                mybir.AluOpType.mult, mybir.AluOpType.add,
            )
            ps_t = psp.tile([128, 512], f32, name="ps", tag="ps")
            nc.tensor.transpose(ps_t[0:1, 0:NLOC], deg_col[:], ident[:])
            deg_row = singles.tile([1, NLOC], f32)
            nc.vector.tensor_copy(deg_row[:], ps_t[0:1, 0:NLOC])

            # ---------------- preprocess: h0 ----------------
            hT = [singles.tile([128, N], f32, name=f"hT{ht}", tag=f"hT{ht}") for ht in range(HT)]
            p1 = [work.tile([128, N], f32, name=f"p1_{ht}", tag=f"p1_{ht}") for ht in range(HT)]
            for ht in range(HT):
                ps = psp.tile([128, 512], f32, name="ps", tag="ps")
                nc.tensor.matmul(
                    ps[:, 0:N], Wpre1_sb[:, ht * 128:(ht + 1) * 128], xT_sb[:],
                    start=True, stop=True,
                )
                nc.scalar.activation(
                    p1[ht][:], ps[:, 0:N], mybir.ActivationFunctionType.Relu,
                    bias=preb1[:, ht:ht + 1],
                )
            for ht in range(HT):
                ps = psp.tile([128, 512], f32, name="ps", tag="ps")
                _mm_acc(nc, ps[:, 0:N], W_pre2, ht * 128, p1, HT)
                nc.scalar.activation(
                    hT[ht][:], ps[:, 0:N], mybir.ActivationFunctionType.Identity,
                    bias=preb2[:, ht:ht + 1],
                )

            # ---------------- message passing iterations ----------------
            for t in range(T):
                # hjbT = (h @ W1_j + b1).T  -> [h, j] layout, bf16
                hjbT = [work.tile([128, N], e_dt, name=f"hjbT{ht}", tag=f"hjbT{ht}") for ht in range(HT)]
                for ht in range(HT):
                    ps = psp.tile([128, 512], f32, name="ps", tag="ps")
                    _mm_acc(nc, ps[:, 0:N], W_m1j, ht * 128, hT, HT)
                    nc.scalar.activation(
                        hjbT[ht][:], ps[:, 0:N],
                        mybir.ActivationFunctionType.Identity,
                        bias=msgb1[:, ht:ht + 1],
                    )
                # hiT = (h_loc @ W1_i).T -> [h, i] layout (f32: TS scalar must be f32)
                hiTf = [work.tile([128, NLOC], f32, name=f"hiTf{ht}", tag=f"hiTf{ht}") for ht in range(HT)]
                for ht in range(HT):
                    ps = psp.tile([128, 512], f32, name="ps", tag="ps")
                    _mm_acc(
                        nc, ps[:, 0:NLOC], W_m1i, ht * 128,
                        [h_[:, 0:NLOC] for h_ in hT], HT,
                    )
                    nc.vector.tensor_copy(hiTf[ht][:], ps[:, 0:NLOC])

                # e-loop. adj_bc holds 32*(adj-1) in {-32, 0}: masked entries
                # die after the relu. Two paths per 16-receiver block:
                #  - ScalarE path (SC_OF_16 units): DVE TT mask-inject of the
                #    block's SC receivers, then ACT Relu+bias+accum per unit.
                #  - fused DVE path (rest): one custom DVE op per unit does
                #    relu(hjbT + adj_bc + hi) with fp32 accum — no mask pass.
                aggT = [work.tile([128, NLOC], f32, name=f"aggT{ht}", tag=f"aggT{ht}") for ht in range(HT)]
                rows_per = NLOC // CH  # receivers per broadcast chunk (16)
                for ht in range(HT):
                    hjb_rep = bass.AP(
                        tensor=hjbT[ht].tensor, offset=hjbT[ht].offset,
                        ap=[hjbT[ht].ap[0], [0, SC_OF_16], [1, N]],
                    )
                    for g in range(0, NLOC, 16):
                        ch = adj_bct[g // rows_per]
                        co = (g % rows_per) * N
                        if SC_OF_16 > 0:
                            w = eloop.tile([128, SC_OF_16 * N], e_dt, name="w", tag="w")
                            nc.vector.tensor_tensor(
                                out=w[:], in0=hjb_rep,
                                in1=ch[:, co:co + SC_OF_16 * N],
                                op=mybir.AluOpType.add,
                            )
                            for k in range(SC_OF_16):
                                i = g + k
                                scr = eloop.tile([128, N], e_dt, name="scr", tag="scr")
                                nc.scalar.activation(
                                    scr[:], w[:, k * N:(k + 1) * N],
                                    mybir.ActivationFunctionType.Relu,
                                    bias=hiTf[ht][:, i:i + 1],
                                    accum_out=aggT[ht][:, i:i + 1],
                                )
                        for k in range(SC_OF_16, 16):
                            i = g + k
                            scr = eloop.tile([128, N], e_dt, name="scr", tag="scr")
                            nc.vector._custom_dve(
                                MSG_RELU_ACC,
                                out=scr[:], in0=hjbT[ht][:],
                                in1=ch[:, co + k * N:co + (k + 1) * N],
                                s0=hiTf[ht][:, i:i + 1], s1=0.0,
                                accum_out=aggT[ht][:, i:i + 1],
                            )

                # msgT = W2.T @ aggT + outer(b2, deg)
                msgT = [work.tile([128, NLOC], f32, name=f"msgT{ht}", tag=f"msgT{ht}") for ht in range(HT)]
                for ht in range(HT):
                    ps = psp.tile([128, 512], f32, name="ps", tag="ps")
                    _mm_acc(
                        nc, ps[:, 0:NLOC], W_m2, ht * 128, aggT, HT,
                        extra=(msgb2[0:1, ht * 128:(ht + 1) * 128], deg_row[:]),
                    )
                    nc.vector.tensor_copy(msgT[ht][:], ps[:, 0:NLOC])

                # GRU gates (torch order r, z, n); everything [gate-dim, i]
                hloc = [h_[:, 0:NLOC] for h_ in hT]
                ps_rz = psp.tile([128, 512], f32, name="ps", tag="ps")
                for mt in range(4):  # r0 r1 z0 z1
                    for kt in range(HT):
                        nc.tensor.matmul(
                            ps_rz[:, mt * 128:(mt + 1) * 128],
                            W_ih[:, kt * 768 + mt * 128: kt * 768 + (mt + 1) * 128],
                            msgT[kt][:], start=(kt == 0), stop=False,
                        )
                    for kt in range(HT):
                        nc.tensor.matmul(
                            ps_rz[:, mt * 128:(mt + 1) * 128],
                            W_hh[:, kt * 768 + mt * 128: kt * 768 + (mt + 1) * 128],
                            hloc[kt], start=False, stop=(kt == HT - 1),
                        )
                rz = work.tile([128, 512], f32, name="rz", tag="rz")
                for mt in range(4):
                    nc.scalar.activation(
                        rz[:, mt * 128:(mt + 1) * 128],
                        ps_rz[:, mt * 128:(mt + 1) * 128],
                        mybir.ActivationFunctionType.Sigmoid,
                        bias=brz[:, mt:mt + 1],
                    )
                # n = tanh(gi_n + bihn + r * (gh_n + bhhn))
                ps_gin = psp.tile([128, 512], f32, name="ps", tag="ps")
                ps_hn = psp.tile([128, 512], f32, name="ps", tag="ps")
                for ht in range(HT):
                    for kt in range(HT):
                        nc.tensor.matmul(
                            ps_gin[:, ht * 128:(ht + 1) * 128],
                            W_ih[:, kt * 768 + (4 + ht) * 128: kt * 768 + (5 + ht) * 128],
                            msgT[kt][:], start=(kt == 0), stop=(kt == HT - 1),
                        )
                        nc.tensor.matmul(
                            ps_hn[:, ht * 128:(ht + 1) * 128],
                            W_hh[:, kt * 768 + (4 + ht) * 128: kt * 768 + (5 + ht) * 128],
                            hloc[kt], start=(kt == 0), stop=(kt == HT - 1),
                        )
                hnew = [work.tile([128, NLOC], f32, name=f"hnew{ht}", tag=f"hnew{ht}") for ht in range(HT)]
                for ht in range(HT):
                    hn = work.tile([128, NLOC], f32, name="hn", tag="hn")
                    nc.scalar.activation(
                        hn[:], ps_hn[:, ht * 128:(ht + 1) * 128],
                        mybir.ActivationFunctionType.Identity,
                        bias=bhhn[:, ht:ht + 1],
                    )
                    rhn = work.tile([128, NLOC], f32, name="rhn", tag="rhn")
                    nc.vector.tensor_mul(rhn[:], rz[:, ht * 128:(ht + 1) * 128], hn[:])
                    nsum = work.tile([128, NLOC], f32, name="nsum", tag="nsum")
                    nc.vector.tensor_add(
                        nsum[:], rhn[:], ps_gin[:, ht * 128:(ht + 1) * 128]
                    )
                    n_t = work.tile([128, NLOC], f32, name="n_t", tag="n_t")
                    nc.scalar.activation(
                        n_t[:], nsum[:], mybir.ActivationFunctionType.Tanh,
                        bias=bihn[:, ht:ht + 1],
                    )
                    # h' = n + z*(h - n)
                    hmn = work.tile([128, NLOC], f32, name="hmn", tag="hmn")
                    nc.vector.tensor_sub(hmn[:], hloc[ht], n_t[:])
                    zh = work.tile([128, NLOC], f32, name="zh", tag="zh")
                    nc.vector.tensor_mul(zh[:], rz[:, 256 + ht * 128: 256 + (ht + 1) * 128], hmn[:])
                    nc.vector.tensor_add(hnew[ht][:], n_t[:], zh[:])

                # exchange: AllReduce(pair) of my update; partner = sum - mine
                for ht in range(HT):
                    nc.sync.dma_start(
                        out=cc_in[t][ht * 128:(ht + 1) * 128, :], in_=hnew[ht][:]
                    )
                nc.gpsimd.collective_compute(
                    "AllReduce", mybir.AluOpType.add, replica_groups=groups,
                    ins=[cc_in[t][:]], outs=[cc_out[t][:]],
                )
                for ht in range(HT):
                    rem = work.tile([128, NLOC], f32, name="rem", tag="rem")
                    nc.sync.dma_start(
                        out=rem[:], in_=cc_out[t][ht * 128:(ht + 1) * 128, :]
                    )
                    nc.vector.tensor_copy(hT[ht][:, 0:NLOC], hnew[ht][:])
                    nc.vector.tensor_sub(hT[ht][:, NLOC:N], rem[:], hnew[ht][:])

            # ---------------- readout ----------------
            gT = [work.tile([128, 1], f32, name=f"gT{ht}", tag=f"gT{ht}") for ht in range(HT)]
            for ht in range(HT):
                nc.vector.reduce_sum(gT[ht][:], hT[ht][:], axis=mybir.AxisListType.X)
            y1 = [work.tile([128, 1], f32, name=f"y1{ht}", tag=f"y1{ht}") for ht in range(HT)]
            for ht in range(HT):
                ps = psp.tile([128, 512], f32, name="ps", tag="ps")
                _mm_acc(nc, ps[:, 0:1], W_ro1, ht * 128, gT, HT)
                nc.scalar.activation(
                    y1[ht][:], ps[:, 0:1], mybir.ActivationFunctionType.Relu,
                    bias=rob1[:, ht:ht + 1],
                )
            ps_q = psp.tile([128, 512], f32, name="ps", tag="ps")
            for kt in range(HT):
                nc.tensor.matmul(
                    ps_q[0:A, 0:1], W_ro2[:, kt * A:(kt + 1) * A], y1[kt][:],
                    start=(kt == 0), stop=(kt == HT - 1),
                )
            q_sb = work.tile([A, 1], f32, name="q_sb", tag="q_sb")
            nc.scalar.activation(
                q_sb[:], ps_q[0:A, 0:1], mybir.ActivationFunctionType.Identity,
                bias=rob2[:],
            )
            nc.sync.dma_start(out=q_out[:], in_=q_sb[:])

    nc.compile()
    return nc


def _in_maps(inputs):
    nf = np.asarray(inputs["node_features"], np.float32)
    adj = np.asarray(inputs["adjacency"])
    msg_W1 = np.asarray(inputs["msg_W1"], np.float32)
    gbih = np.asarray(inputs["gru_bih"], np.float32)
    gbhh = np.asarray(inputs["gru_bhh"], np.float32)

    def cols(v, nt):  # [nt*128] -> [128, nt] partition-major columns
        return np.ascontiguousarray(np.asarray(v, np.float32).reshape(nt, 128).T)

    shared = {
        "pre_W1": np.asarray(inputs["pre_W1"], np.float32),
        "pre_W2": np.asarray(inputs["pre_W2"], np.float32),
        "W1i": np.ascontiguousarray(msg_W1[:H]),
        "W1j": np.ascontiguousarray(msg_W1[H:]),
        "W2m": np.asarray(inputs["msg_W2"], np.float32),
        "Wih": np.asarray(inputs["gru_Wih"], np.float32),
        "Whh": np.asarray(inputs["gru_Whh"], np.float32),
        "roW1": np.asarray(inputs["ro_W1"], np.float32),
        "roW2": np.asarray(inputs["ro_W2"], np.float32),
        "preb1c": cols(inputs["pre_b1"], HT),
        "preb2c": cols(inputs["pre_b2"], HT),
        "msgb1c": cols(inputs["msg_b1"], HT),
        "msgb2r": np.asarray(inputs["msg_b2"], np.float32)[None, :],
        "brzc": cols((gbih + gbhh)[: 2 * H], 4),
        "bihnc": cols(gbih[2 * H:], HT),
        "bhhnc": cols(gbhh[2 * H:], HT),
        "rob1c": cols(inputs["ro_b1"], HT),
        "rob2c": np.asarray(inputs["ro_b2"], np.float32)[:, None],
        "ident": np.eye(128, dtype=np.float32),
    }
    maps = []
    for c in range(8):
        b, half = c // 2, c % 2
        lo, hi = half * NLOC, (half + 1) * NLOC
        perm = np.r_[lo:hi, 0:lo, hi:N]
        m = dict(shared)
        m["xT"] = np.ascontiguousarray(nf[b].T[:, perm])
        m["adjb"] = np.ascontiguousarray(
            ((adj[b, lo:hi][:, perm] - 1) * 32).astype(BF16_NP)
        )
        maps.append(m)
    return maps


def kernel(**inputs) -> np.ndarray:
    if "nc" not in _CACHE:
        _CACHE["nc"] = build_program()
    nc = _CACHE["nc"]
    maps = _in_maps(inputs)
    res = run_bass_kernel_spmd(nc, maps, list(range(8))).results
    q = np.stack([res[2 * b]["q_out"][:, 0] for b in range(B)]).astype(np.float32)
    return q

